# revision 21
# baseline (speedup 1.0000x reference)
"""Cross-attention kernel for Trainium2 (Bass/Tile), 8-core data-parallel over batch.

Per core (one batch element):
  q1 = x1 @ Wq + bq ; k2 = x2 @ Wk + bk ; v2 = x2 @ Wv + bv
  out = softmax(q1 @ k2^T / sqrt(D)) @ v2

Measured-HW design (diverges from the naive cost model):
  - A self-loading matmul whose stationary CHANGES pays ~44ns; re-using the
    previous stationary runs at the pure row rate (~0.42 ns/row).  Every
    phase is therefore built from PAIRS of 512-wide matmuls sharing one
    stationary (1024-wide chunks, two PSUM banks per pair).
  - All matmul operands bf16 (mixed dtypes are rejected by neuronxcc).
    GpSimd-issued DMAs convert dtypes in flight, so x rows and weights are
    cast-DMA'd f32->bf16 directly; no staging or engine casts.
  - x1/x2 transposed by the DMA XBAR (dma_start_transpose, 2-byte):
    [128,1024] -> [128,8,128] e-major, exactly the [d%128, d//128, s]
    layout the projections need.  Zero PE/DVE transpose cost.
  - scoresT[k, q] = k2T-tile^T @ q1T on PE; exp on ACT (logits ~ N(0,1),
    no max subtraction), fused 1/sqrt(D) scale, bf16 out.
  - PV uses triples per (qt, kt) stationary: dh0, dh1 and the 8-wide
    ones-column denominator matmul (its weight loads are free).
    Normalization fused into the DVE evacuation; bv folded into v2.
  - Engine roles: PE = matmuls only; ACT = exp + q1T/k2T bias evacs;
    DVE = v2 evacs + normalize + reciprocal; GpSimd = all casting DMAs +
    output stores; Sync = XBAR transposes.
  - PSUM: 7-buf shared pool for paired groups + 1 denominator bank.
"""

import sys

for _p in ("/root/.axon_site", "/root/.axon_site/_ro/trn_rl_repo",
           "/root/.axon_site/_ro/pypackages", "/opt/trn_rl_repo", "/opt/pypackages"):
    if _p not in sys.path:
        sys.path.append(_p)

import numpy as np

import concourse.bass as bass
import concourse.mybir as mybir
import concourse.tile as tile
from concourse import bacc
from concourse.bass_utils import run_bass_kernel_spmd

F32 = mybir.dt.float32
F32R = mybir.dt.float32r
BF16 = mybir.dt.bfloat16

P = 128
HW = 512         # half-width: PSUM bank width (f32) = moving dim per matmul
CW = 1024        # chunk width (queries or keys per paired phase)
N_CORES = 8

IDENT = mybir.ActivationFunctionType.Identity
EXP = mybir.ActivationFunctionType.Exp


def build(S=2048, D=1024, scale=None):
    """Original direct kernel (handles arbitrary biases)."""
    assert S % CW == 0 and D % P == 0
    n_st = S // P
    n_dt = D // P
    n_cw = S // CW
    n_qt = CW // P
    if scale is None:
        scale = 1.0 / float(np.sqrt(D).astype(np.float32))

    nc = bacc.Bacc("TRN2", target_bir_lowering=False, debug=False)

    x1 = nc.dram_tensor("x1", [S, D], F32, kind="ExternalInput").ap()
    x2 = nc.dram_tensor("x2", [S, D], F32, kind="ExternalInput").ap()
    Wq = nc.dram_tensor("Wq", [D, D], F32, kind="ExternalInput").ap()
    bq = nc.dram_tensor("bq", [D], F32, kind="ExternalInput").ap()
    Wk = nc.dram_tensor("Wk", [D, D], F32, kind="ExternalInput").ap()
    bk = nc.dram_tensor("bk", [D], F32, kind="ExternalInput").ap()
    Wv = nc.dram_tensor("Wv", [D, D], F32, kind="ExternalInput").ap()
    bv = nc.dram_tensor("bv", [D], F32, kind="ExternalInput").ap()
    out = nc.dram_tensor("out", [S, D], F32, kind="ExternalOutput").ap()

    out_r = out.rearrange("(t p) d -> p t d", p=P)
    Wq_r = Wq.rearrange("(a p) e -> p a e", p=P)
    Wk_r = Wk.rearrange("(a p) e -> p a e", p=P)
    Wv_r = Wv.rearrange("(a p) d -> p a d", p=P)

    with tile.TileContext(nc) as tc:
        with (
            tc.tile_pool(name="const", bufs=1) as p_const,
            tc.tile_pool(name="big", bufs=1) as p_big,
            tc.tile_pool(name="xnb", bufs=1) as p_xnb,
            tc.tile_pool(name="xn", bufs=3) as p_xn,
            tc.tile_pool(name="xt", bufs=1) as p_xt,
            tc.tile_pool(name="o", bufs=2) as p_o,
            tc.tile_pool(name="wq0", bufs=1) as p_wq0,
            tc.tile_pool(name="stat", bufs=2) as p_stat,
            tc.tile_pool(name="pp", bufs=5, space=bass.MemorySpace.PSUM) as pp,
            tc.tile_pool(name="psd", bufs=1, space=bass.MemorySpace.PSUM) as psd_p,
            tc.tile_pool(name="ptr", bufs=2, space=bass.MemorySpace.PSUM) as ptr,
        ):
            from concourse.masks import make_identity
            ident_ft = p_const.tile([P, P], F32)
            make_identity(nc, ident_ft[:])
            ident_rt = p_const.tile([P, P], F32R)
            nc.vector.tensor_copy(ident_rt[:], ident_ft[:])
            ident_r = ident_rt[:]
            cpack = p_const.tile([P, 2 * n_dt], F32)
            bq_sb = cpack[:, 0:n_dt]
            nc.gpsimd.dma_start(out=bq_sb, in_=bq.rearrange("(a p) -> p a", p=P))
            bk_sb = cpack[:, n_dt:2 * n_dt]
            nc.gpsimd.dma_start(out=bk_sb, in_=bk.rearrange("(a p) -> p a", p=P))
            ones_bf = p_const.tile([P, 8], BF16)
            nc.gpsimd.memset(ones_bf[:], 1.0)
            bv_bc = p_const.tile([P, D], F32)
            nc.gpsimd.dma_start(
                out=bv_bc[:],
                in_=bv.rearrange("(a d) -> a d", a=1).broadcast_to([P, D]))

            k2t = p_big.tile([P, n_dt, S], BF16, tag="k2t")
            v2 = p_big.tile([P, n_st, D], BF16, tag="v2")

            def cast_rows(x_ap, s0):
                xnb = p_xnb.tile([P, CW // P, D], BF16, tag="xnb", name="xnb")
                for st in range(CW // P):
                    nc.gpsimd.dma_start(
                        out=xnb[:, st, :],
                        in_=x_ap[s0 + st * P:s0 + (st + 1) * P, :])
                return xnb

            def xbar_tr(xnb, xt, c0=0):
                for st in range(CW // P):
                    nc.sync.dma_start_transpose(
                        out=xt[:, :, c0 + st * P:c0 + (st + 1) * P],
                        in_=xnb[:, st, :])

            def pe_tr(x_ap, s0, xt, c0, sts=None):
                for st in (range(CW // P) if sts is None else sts):
                    xn = p_xn.tile([P, D], F32R, tag="xn", name="xn")
                    nc.scalar.dma_start(
                        out=xn[:],
                        in_=x_ap[s0 + st * P:s0 + (st + 1) * P, :].bitcast(F32R))
                    for half in range(2):
                        tr = ptr.tile([P, HW], F32, tag="tr", name="tr")
                        for dsub in range(4):
                            d0 = (half * 4 + dsub) * P
                            nc.tensor.transpose(
                                tr[:, dsub * P:(dsub + 1) * P].bitcast(F32R),
                                xn[:, d0:d0 + P], ident_r)
                        dst = xt[:, half * 4:(half + 1) * 4,
                                 c0 + st * P:c0 + (st + 1) * P]
                        nc.vector.tensor_copy(
                            dst, tr[:].rearrange("p (a b) -> p a b", a=4))

            def load_w_bf16(w_r, dst):
                for blk in range(n_dt):
                    nc.gpsimd.dma_start(
                        out=dst[:, :, blk * P:(blk + 1) * P],
                        in_=w_r[:, :, blk * P:(blk + 1) * P])

            def load_w_half(w_r, dst, h):
                for j in range(4):
                    blk = h * 4 + j
                    nc.gpsimd.dma_start(
                        out=dst[:, :, j * P:(j + 1) * P],
                        in_=w_r[:, :, blk * P:(blk + 1) * P])

            with (
                tc.tile_pool(name="x2t", bufs=1) as p_x2t,
                tc.tile_pool(name="wkv", bufs=1) as p_wkv,
            ):
                x2t_a = p_x2t.tile([P, n_dt, CW], BF16, tag="x2ta", name="x2ta")
                x2t_b = p_x2t.tile([P, n_dt, CW], BF16, tag="x2tb", name="x2tb")
                pe_tr(x2, 0, x2t_a, 0, sts=range(0, 4))
                wk_bf = p_wkv.tile([P, n_dt, D], BF16, tag="wk")
                load_w_bf16(Wk_r, wk_bf)
                wv_bf = p_wkv.tile([P, n_dt, D], BF16, tag="wv")
                load_w_bf16(Wv_r, wv_bf)
                wq0_h0 = p_wq0.tile([P, n_dt, HW], BF16, tag="wq0")
                load_w_half(Wq_r, wq0_h0, 0)
                xnb = cast_rows(x2, CW)
                xbar_tr(xnb, x2t_b, 0)
                xnb = cast_rows(x1, 0)
                x1t = p_xt.tile([P, n_dt, CW], BF16, tag="xt", name="x1t")
                xbar_tr(xnb, x1t)

                for kp in range(n_cw):
                    x2t = (x2t_a if kp == 0 else x2t_b)[:, :, :]
                    if kp == 0:
                        # K-kp0 split into half-phases: the pa half needs only
                        # x2t_a tiles 0-3, so PE starts ~12us earlier and the
                        # remaining transposes hide behind matmul work.
                        for et in range(n_dt):
                            pa = pp.tile([P, HW], F32, tag="ps", name="psA")
                            for dt in range(n_dt):
                                st_ap = wk_bf[:, dt, et * P:(et + 1) * P]
                                nc.tensor.matmul(pa[:], st_ap, x2t[:, dt, 0:HW],
                                                 start=(dt == 0),
                                                 stop=(dt == n_dt - 1))
                            nc.scalar.activation(
                                k2t[:, et, 0:HW], pa[:], IDENT,
                                bias=bk_sb[:, et:et + 1], scale=1.0)
                            # interleave the remaining transposes: each one
                            # unblocks the next xn-row DMA (pool rotation), so
                            # tiles 4-7 finish before the pb half needs them
                            if 1 <= et <= 4:
                                pe_tr(x2, 0, x2t_a, 0, sts=[et + 3])
                        for et in range(n_dt):
                            pb = pp.tile([P, HW], F32, tag="ps", name="psB")
                            for dt in range(n_dt):
                                st_ap = wk_bf[:, dt, et * P:(et + 1) * P]
                                nc.tensor.matmul(pb[:], st_ap, x2t[:, dt, HW:CW],
                                                 start=(dt == 0),
                                                 stop=(dt == n_dt - 1))
                            nc.scalar.activation(
                                k2t[:, et, HW:CW], pb[:], IDENT,
                                bias=bk_sb[:, et:et + 1], scale=1.0)
                        continue_k = True
                    else:
                        for et in range(n_dt):
                            pa = pp.tile([P, HW], F32, tag="ps", name="psA")
                            pb = pp.tile([P, HW], F32, tag="ps", name="psB")
                            for dt in range(n_dt):
                                st_ap = wk_bf[:, dt, et * P:(et + 1) * P]
                                nc.tensor.matmul(pa[:], st_ap, x2t[:, dt, 0:HW],
                                                 start=(dt == 0),
                                                 stop=(dt == n_dt - 1))
                                nc.tensor.matmul(pb[:], st_ap, x2t[:, dt, HW:CW],
                                                 start=(dt == 0),
                                                 stop=(dt == n_dt - 1))
                            for half, ps in ((0, pa), (1, pb)):
                                nc.scalar.activation(
                                    k2t[:, et, kp * CW + half * HW:
                                        kp * CW + (half + 1) * HW],
                                    ps[:], IDENT, bias=bk_sb[:, et:et + 1],
                                    scale=1.0)
                    for kt in range(CW // P):
                        pa = pp.tile([P, HW], F32, tag="ps", name="psA")
                        pb = pp.tile([P, HW], F32, tag="ps", name="psB")
                        for dt in range(n_dt):
                            st_ap = x2t[:, dt, kt * P:(kt + 1) * P]
                            nc.tensor.matmul(pa[:], st_ap, wv_bf[:, dt, 0:HW],
                                             start=(dt == 0), stop=(dt == n_dt - 1))
                            nc.tensor.matmul(pb[:], st_ap, wv_bf[:, dt, HW:CW],
                                             start=(dt == 0), stop=(dt == n_dt - 1))
                        ktg = kp * (CW // P) + kt
                        for half, ps in ((0, pa), (1, pb)):
                            nc.vector.tensor_tensor(
                                out=v2[:, ktg, half * HW:(half + 1) * HW],
                                in0=ps[:], in1=bv_bc[:, half * HW:(half + 1) * HW],
                                op=mybir.AluOpType.add)

            with (
                tc.tile_pool(name="qe", bufs=1) as p_qe,
                tc.tile_pool(name="wq", bufs=3) as p_wq,
            ):
                q1t = p_qe.tile([P, n_dt, CW], BF16, tag="q1t")
                expT = p_qe.tile([P, n_st, CW], BF16, tag="expT")
                wq_h = [wq0_h0, None]
                wq_h[1] = p_wq.tile([P, n_dt, HW], BF16, tag="wq", name="wqh1")
                load_w_half(Wq_r, wq_h[1], 1)
                for c in range(n_cw):
                    last = c + 1 >= n_cw
                    xnb = None if last else cast_rows(x1, (c + 1) * CW)
                    for et in range(n_dt):
                        pa = pp.tile([P, HW], F32, tag="ps", name="psA")
                        pb = pp.tile([P, HW], F32, tag="ps", name="psB")
                        wqh = wq_h[et // 4]
                        ec = et % 4
                        for dt in range(n_dt):
                            st_ap = wqh[:, dt, ec * P:(ec + 1) * P]
                            nc.tensor.matmul(pa[:], st_ap, x1t[:, dt, 0:HW],
                                             start=(dt == 0), stop=(dt == n_dt - 1))
                            nc.tensor.matmul(pb[:], st_ap, x1t[:, dt, HW:CW],
                                             start=(dt == 0), stop=(dt == n_dt - 1))
                        for half, ps in ((0, pa), (1, pb)):
                            nc.scalar.activation(
                                q1t[:, et, half * HW:(half + 1) * HW], ps[:],
                                IDENT, bias=bq_sb[:, et:et + 1], scale=1.0)
                    if not last:
                        x1t = p_xt.tile([P, n_dt, CW], BF16, tag="xt",
                                        name="x1t")
                        xbar_tr(xnb, x1t)
                    for kt in range(n_st):
                        pa = pp.tile([P, HW], F32, tag="ps", name="psA")
                        pb = pp.tile([P, HW], F32, tag="ps", name="psB")
                        for et in range(n_dt):
                            st_ap = k2t[:, et, kt * P:(kt + 1) * P]
                            nc.tensor.matmul(pa[:], st_ap, q1t[:, et, 0:HW],
                                             start=(et == 0), stop=(et == n_dt - 1))
                            nc.tensor.matmul(pb[:], st_ap, q1t[:, et, HW:CW],
                                             start=(et == 0), stop=(et == n_dt - 1))
                        nc.scalar.activation(expT[:, kt, 0:HW], pa[:], EXP,
                                             bias=0.0, scale=scale)
                        nc.scalar.activation(expT[:, kt, HW:CW], pb[:], EXP,
                                             bias=0.0, scale=scale)
                        if kt == 7 and not last:
                            wq_h[0] = p_wq.tile([P, n_dt, HW], BF16,
                                                tag="wq", name="wqh0")
                            load_w_half(Wq_r, wq_h[0], 0)
                            wq_h[1] = p_wq.tile([P, n_dt, HW], BF16,
                                                tag="wq", name="wqh1")
                            load_w_half(Wq_r, wq_h[1], 1)
                    for qt in range(n_qt):
                        qs = slice(qt * P, (qt + 1) * P)
                        qt_g = c * n_qt + qt
                        pa = pp.tile([P, HW], F32, tag="ps", name="psA")
                        pb = pp.tile([P, HW], F32, tag="ps", name="psB")
                        pd = psd_p.tile([P, 8], F32, tag="psd", name="psd")
                        for kt in range(n_st):
                            st_ap = expT[:, kt, qs]
                            nc.tensor.matmul(pd[:], st_ap, ones_bf[:],
                                             start=(kt == 0), stop=(kt == n_st - 1))
                            nc.tensor.matmul(pa[:], st_ap, v2[:, kt, 0:HW],
                                             start=(kt == 0), stop=(kt == n_st - 1))
                            nc.tensor.matmul(pb[:], st_ap, v2[:, kt, HW:CW],
                                             start=(kt == 0), stop=(kt == n_st - 1))
                        rden = p_stat.tile([P, 1], F32, tag="rden", name="rden")
                        nc.vector.reciprocal(rden[:], pd[:, 0:1])
                        for half, ps in ((0, pa), (1, pb)):
                            osb = p_o.tile([P, HW], F32, tag="osb", name="osb")
                            nc.vector.tensor_scalar_mul(osb[:], ps[:],
                                                        rden[:, 0:1])
                            nc.gpsimd.dma_start(
                                out=out_r[:, qt_g, half * HW:(half + 1) * HW],
                                in_=osb[:])

    nc.compile()
    return nc


_NC_CACHE = {}


def _get_nc(S, D):
    if (S, D) not in _NC_CACHE:
        _NC_CACHE[(S, D)] = build(S, D)
    return _NC_CACHE[(S, D)]


def kernel(x1, x2, Wq, bq, Wk, bk, Wv, bv):
    B, S, D = x1.shape
    assert (B, S, D) == (8, 2048, 1024), (B, S, D)
    nc = _get_nc(S, D)
    f = np.float32
    shared = {
        "Wq": np.ascontiguousarray(Wq, f), "bq": np.ascontiguousarray(bq, f),
        "Wk": np.ascontiguousarray(Wk, f), "bk": np.ascontiguousarray(bk, f),
        "Wv": np.ascontiguousarray(Wv, f), "bv": np.ascontiguousarray(bv, f),
    }
    in_maps = [
        dict(x1=np.ascontiguousarray(x1[b], f),
             x2=np.ascontiguousarray(x2[b], f), **shared)
        for b in range(N_CORES)
    ]
    res = run_bass_kernel_spmd(nc, in_maps, list(range(N_CORES))).results
    return np.stack([res[b]["out"] for b in range(N_CORES)], axis=0).astype(f)


# revision 23
# speedup vs baseline: 1.0123x; 1.0123x over previous
"""Cross-attention kernel for Trainium2 (Bass/Tile), 8-core data-parallel over batch.

Per core (one batch element):
  q1 = x1 @ Wq + bq ; k2 = x2 @ Wk + bk ; v2 = x2 @ Wv + bv
  out = softmax(q1 @ k2^T / sqrt(D)) @ v2

Measured-HW design (diverges from the naive cost model):
  - A self-loading matmul whose stationary CHANGES pays ~44ns; re-using the
    previous stationary runs at the pure row rate (~0.42 ns/row).  Every
    phase is therefore built from PAIRS of 512-wide matmuls sharing one
    stationary (1024-wide chunks, two PSUM banks per pair).
  - All matmul operands bf16 (mixed dtypes are rejected by neuronxcc).
    GpSimd-issued DMAs convert dtypes in flight, so x rows and weights are
    cast-DMA'd f32->bf16 directly; no staging or engine casts.
  - x1/x2 transposed by the DMA XBAR (dma_start_transpose, 2-byte):
    [128,1024] -> [128,8,128] e-major, exactly the [d%128, d//128, s]
    layout the projections need.  Zero PE/DVE transpose cost.
  - scoresT[k, q] = k2T-tile^T @ q1T on PE; exp on ACT (logits ~ N(0,1),
    no max subtraction), fused 1/sqrt(D) scale, bf16 out.
  - PV uses triples per (qt, kt) stationary: dh0, dh1 and the 8-wide
    ones-column denominator matmul (its weight loads are free).
    Normalization fused into the DVE evacuation; bv folded into v2.
  - Engine roles: PE = matmuls only; ACT = exp + q1T/k2T bias evacs;
    DVE = v2 evacs + normalize + reciprocal; GpSimd = all casting DMAs +
    output stores; Sync = XBAR transposes.
  - PSUM: 7-buf shared pool for paired groups + 1 denominator bank.
"""

import sys

for _p in ("/root/.axon_site", "/root/.axon_site/_ro/trn_rl_repo",
           "/root/.axon_site/_ro/pypackages", "/opt/trn_rl_repo", "/opt/pypackages"):
    if _p not in sys.path:
        sys.path.append(_p)

import numpy as np

import concourse.bass as bass
import concourse.mybir as mybir
import concourse.tile as tile
from concourse import bacc
from concourse.bass_utils import run_bass_kernel_spmd

F32 = mybir.dt.float32
F32R = mybir.dt.float32r
BF16 = mybir.dt.bfloat16

P = 128
HW = 512         # half-width: PSUM bank width (f32) = moving dim per matmul
CW = 1024        # chunk width (queries or keys per paired phase)
N_CORES = 8

IDENT = mybir.ActivationFunctionType.Identity
EXP = mybir.ActivationFunctionType.Exp


def build(S=2048, D=1024, scale=None):
    """Original direct kernel (handles arbitrary biases)."""
    assert S % CW == 0 and D % P == 0
    n_st = S // P
    n_dt = D // P
    n_cw = S // CW
    n_qt = CW // P
    if scale is None:
        scale = 1.0 / float(np.sqrt(D).astype(np.float32))

    nc = bacc.Bacc("TRN2", target_bir_lowering=False, debug=False)

    x1 = nc.dram_tensor("x1", [S, D], F32, kind="ExternalInput").ap()
    x2 = nc.dram_tensor("x2", [S, D], F32, kind="ExternalInput").ap()
    Wq = nc.dram_tensor("Wq", [D, D], F32, kind="ExternalInput").ap()
    bq = nc.dram_tensor("bq", [D], F32, kind="ExternalInput").ap()
    Wk = nc.dram_tensor("Wk", [D, D], F32, kind="ExternalInput").ap()
    bk = nc.dram_tensor("bk", [D], F32, kind="ExternalInput").ap()
    Wv = nc.dram_tensor("Wv", [D, D], F32, kind="ExternalInput").ap()
    bv = nc.dram_tensor("bv", [D], F32, kind="ExternalInput").ap()
    out = nc.dram_tensor("out", [S, D], F32, kind="ExternalOutput").ap()

    out_r = out.rearrange("(t p) d -> p t d", p=P)
    Wq_r = Wq.rearrange("(a p) e -> p a e", p=P)
    Wk_r = Wk.rearrange("(a p) e -> p a e", p=P)
    Wv_r = Wv.rearrange("(a p) d -> p a d", p=P)

    with tile.TileContext(nc) as tc:
        with (
            tc.tile_pool(name="const", bufs=1) as p_const,
            tc.tile_pool(name="big", bufs=1) as p_big,
            tc.tile_pool(name="xnb", bufs=1) as p_xnb,
            tc.tile_pool(name="xn", bufs=4) as p_xn,
            tc.tile_pool(name="xt", bufs=1) as p_xt,
            tc.tile_pool(name="o", bufs=2) as p_o,
            tc.tile_pool(name="wq0", bufs=1) as p_wq0,
            tc.tile_pool(name="stat", bufs=2) as p_stat,
            tc.tile_pool(name="pp", bufs=5, space=bass.MemorySpace.PSUM) as pp,
            tc.tile_pool(name="psd", bufs=1, space=bass.MemorySpace.PSUM) as psd_p,
            tc.tile_pool(name="ptr", bufs=2, space=bass.MemorySpace.PSUM) as ptr,
        ):
            from concourse.masks import make_identity
            ident_ft = p_const.tile([P, P], F32)
            make_identity(nc, ident_ft[:])
            ident_rt = p_const.tile([P, P], F32R)
            nc.vector.tensor_copy(ident_rt[:], ident_ft[:])
            ident_r = ident_rt[:]
            cpack = p_const.tile([P, 2 * n_dt], F32)
            bq_sb = cpack[:, 0:n_dt]
            nc.gpsimd.dma_start(out=bq_sb, in_=bq.rearrange("(a p) -> p a", p=P))
            bk_sb = cpack[:, n_dt:2 * n_dt]
            nc.gpsimd.dma_start(out=bk_sb, in_=bk.rearrange("(a p) -> p a", p=P))
            ones_bf = p_const.tile([P, 8], BF16)
            nc.gpsimd.memset(ones_bf[:], 1.0)
            bv_bc = p_const.tile([P, D], F32)
            nc.gpsimd.dma_start(
                out=bv_bc[:],
                in_=bv.rearrange("(a d) -> a d", a=1).broadcast_to([P, D]))

            k2t = p_big.tile([P, n_dt, S], BF16, tag="k2t")
            v2 = p_big.tile([P, n_st, D], BF16, tag="v2")

            def cast_rows(x_ap, s0):
                xnb = p_xnb.tile([P, CW // P, D], BF16, tag="xnb", name="xnb")
                for st in range(CW // P):
                    nc.gpsimd.dma_start(
                        out=xnb[:, st, :],
                        in_=x_ap[s0 + st * P:s0 + (st + 1) * P, :])
                return xnb

            def xbar_tr(xnb, xt, c0=0):
                for st in range(CW // P):
                    nc.sync.dma_start_transpose(
                        out=xt[:, :, c0 + st * P:c0 + (st + 1) * P],
                        in_=xnb[:, st, :])

            def xn_dma(x_ap, s0, st):
                """Issue one transpose-source row DMA on the scalar queue."""
                xn = p_xn.tile([P, D], F32R, tag="xn", name="xn")
                nc.scalar.dma_start(
                    out=xn[:],
                    in_=x_ap[s0 + st * P:s0 + (st + 1) * P, :].bitcast(F32R))
                return xn

            def xn_tr(xn, st, xt, c0=0):
                """PE-transpose one staged row tile into xt."""
                for half in range(2):
                    tr = ptr.tile([P, HW], F32, tag="tr", name="tr")
                    for dsub in range(4):
                        d0 = (half * 4 + dsub) * P
                        nc.tensor.transpose(
                            tr[:, dsub * P:(dsub + 1) * P].bitcast(F32R),
                            xn[:, d0:d0 + P], ident_r)
                    dst = xt[:, half * 4:(half + 1) * 4,
                             c0 + st * P:c0 + (st + 1) * P]
                    nc.vector.tensor_copy(
                        dst, tr[:].rearrange("p (a b) -> p a b", a=4))

            def load_w_bf16(w_r, dst):
                for blk in range(n_dt):
                    nc.gpsimd.dma_start(
                        out=dst[:, :, blk * P:(blk + 1) * P],
                        in_=w_r[:, :, blk * P:(blk + 1) * P])

            def load_w_half(w_r, dst, h):
                for j in range(4):
                    blk = h * 4 + j
                    nc.gpsimd.dma_start(
                        out=dst[:, :, j * P:(j + 1) * P],
                        in_=w_r[:, :, blk * P:(blk + 1) * P])

            with (
                tc.tile_pool(name="x2t", bufs=1) as p_x2t,
                tc.tile_pool(name="wkv", bufs=1) as p_wkv,
            ):
                x2t_a = p_x2t.tile([P, n_dt, CW], BF16, tag="x2ta", name="x2ta")
                x2t_b = p_x2t.tile([P, n_dt, CW], BF16, tag="x2tb", name="x2tb")
                xns_a = [xn_dma(x2, 0, st) for st in range(0, 4)]
                for st in range(0, 4):
                    xn_tr(xns_a[st], st, x2t_a)
                xns_b = [xn_dma(x2, 0, st) for st in range(4, 8)]
                wk_bf = p_wkv.tile([P, n_dt, D], BF16, tag="wk")
                load_w_bf16(Wk_r, wk_bf)
                wv_bf = p_wkv.tile([P, n_dt, D], BF16, tag="wv")
                load_w_bf16(Wv_r, wv_bf)
                wq0_h0 = p_wq0.tile([P, n_dt, HW], BF16, tag="wq0")
                load_w_half(Wq_r, wq0_h0, 0)
                xnb = cast_rows(x2, CW)
                xbar_tr(xnb, x2t_b, 0)
                xnb = cast_rows(x1, 0)
                x1t = p_xt.tile([P, n_dt, CW], BF16, tag="xt", name="x1t")
                xbar_tr(xnb, x1t)

                for kp in range(n_cw):
                    x2t = (x2t_a if kp == 0 else x2t_b)[:, :, :]
                    if kp == 0:
                        # K-kp0 split into half-phases: the pa half needs only
                        # x2t_a tiles 0-3, so PE starts ~12us earlier and the
                        # remaining transposes hide behind matmul work.
                        for et in range(n_dt):
                            pa = pp.tile([P, HW], F32, tag="ps", name="psA")
                            for dt in range(n_dt):
                                st_ap = wk_bf[:, dt, et * P:(et + 1) * P]
                                nc.tensor.matmul(pa[:], st_ap, x2t[:, dt, 0:HW],
                                                 start=(dt == 0),
                                                 stop=(dt == n_dt - 1))
                            nc.scalar.activation(
                                k2t[:, et, 0:HW], pa[:], IDENT,
                                bias=bk_sb[:, et:et + 1], scale=1.0)
                        for st in range(4, 8):
                            xn_tr(xns_b[st - 4], st, x2t_a)
                        for et in range(n_dt):
                            pb = pp.tile([P, HW], F32, tag="ps", name="psB")
                            for dt in range(n_dt):
                                st_ap = wk_bf[:, dt, et * P:(et + 1) * P]
                                nc.tensor.matmul(pb[:], st_ap, x2t[:, dt, HW:CW],
                                                 start=(dt == 0),
                                                 stop=(dt == n_dt - 1))
                            nc.scalar.activation(
                                k2t[:, et, HW:CW], pb[:], IDENT,
                                bias=bk_sb[:, et:et + 1], scale=1.0)
                        continue_k = True
                    else:
                        for et in range(n_dt):
                            pa = pp.tile([P, HW], F32, tag="ps", name="psA")
                            pb = pp.tile([P, HW], F32, tag="ps", name="psB")
                            for dt in range(n_dt):
                                st_ap = wk_bf[:, dt, et * P:(et + 1) * P]
                                nc.tensor.matmul(pa[:], st_ap, x2t[:, dt, 0:HW],
                                                 start=(dt == 0),
                                                 stop=(dt == n_dt - 1))
                                nc.tensor.matmul(pb[:], st_ap, x2t[:, dt, HW:CW],
                                                 start=(dt == 0),
                                                 stop=(dt == n_dt - 1))
                            for half, ps in ((0, pa), (1, pb)):
                                nc.scalar.activation(
                                    k2t[:, et, kp * CW + half * HW:
                                        kp * CW + (half + 1) * HW],
                                    ps[:], IDENT, bias=bk_sb[:, et:et + 1],
                                    scale=1.0)
                    for kt in range(CW // P):
                        pa = pp.tile([P, HW], F32, tag="ps", name="psA")
                        pb = pp.tile([P, HW], F32, tag="ps", name="psB")
                        for dt in range(n_dt):
                            st_ap = x2t[:, dt, kt * P:(kt + 1) * P]
                            nc.tensor.matmul(pa[:], st_ap, wv_bf[:, dt, 0:HW],
                                             start=(dt == 0), stop=(dt == n_dt - 1))
                            nc.tensor.matmul(pb[:], st_ap, wv_bf[:, dt, HW:CW],
                                             start=(dt == 0), stop=(dt == n_dt - 1))
                        ktg = kp * (CW // P) + kt
                        for half, ps in ((0, pa), (1, pb)):
                            nc.vector.tensor_tensor(
                                out=v2[:, ktg, half * HW:(half + 1) * HW],
                                in0=ps[:], in1=bv_bc[:, half * HW:(half + 1) * HW],
                                op=mybir.AluOpType.add)

            with (
                tc.tile_pool(name="qe", bufs=1) as p_qe,
                tc.tile_pool(name="wq", bufs=3) as p_wq,
            ):
                q1t = p_qe.tile([P, n_dt, CW], BF16, tag="q1t")
                expT = p_qe.tile([P, n_st, CW], BF16, tag="expT")
                wq_h = [wq0_h0, None]
                wq_h[1] = p_wq.tile([P, n_dt, HW], BF16, tag="wq", name="wqh1")
                load_w_half(Wq_r, wq_h[1], 1)
                for c in range(n_cw):
                    last = c + 1 >= n_cw
                    xnb = None if last else cast_rows(x1, (c + 1) * CW)
                    for et in range(n_dt):
                        pa = pp.tile([P, HW], F32, tag="ps", name="psA")
                        pb = pp.tile([P, HW], F32, tag="ps", name="psB")
                        wqh = wq_h[et // 4]
                        ec = et % 4
                        for dt in range(n_dt):
                            st_ap = wqh[:, dt, ec * P:(ec + 1) * P]
                            nc.tensor.matmul(pa[:], st_ap, x1t[:, dt, 0:HW],
                                             start=(dt == 0), stop=(dt == n_dt - 1))
                            nc.tensor.matmul(pb[:], st_ap, x1t[:, dt, HW:CW],
                                             start=(dt == 0), stop=(dt == n_dt - 1))
                        for half, ps in ((0, pa), (1, pb)):
                            nc.scalar.activation(
                                q1t[:, et, half * HW:(half + 1) * HW], ps[:],
                                IDENT, bias=bq_sb[:, et:et + 1], scale=1.0)
                    if not last:
                        x1t = p_xt.tile([P, n_dt, CW], BF16, tag="xt",
                                        name="x1t")
                        xbar_tr(xnb, x1t)
                    for kt in range(n_st):
                        pa = pp.tile([P, HW], F32, tag="ps", name="psA")
                        pb = pp.tile([P, HW], F32, tag="ps", name="psB")
                        for et in range(n_dt):
                            st_ap = k2t[:, et, kt * P:(kt + 1) * P]
                            nc.tensor.matmul(pa[:], st_ap, q1t[:, et, 0:HW],
                                             start=(et == 0), stop=(et == n_dt - 1))
                            nc.tensor.matmul(pb[:], st_ap, q1t[:, et, HW:CW],
                                             start=(et == 0), stop=(et == n_dt - 1))
                        nc.scalar.activation(expT[:, kt, 0:HW], pa[:], EXP,
                                             bias=0.0, scale=scale)
                        nc.scalar.activation(expT[:, kt, HW:CW], pb[:], EXP,
                                             bias=0.0, scale=scale)
                        if kt == 7 and not last:
                            wq_h[0] = p_wq.tile([P, n_dt, HW], BF16,
                                                tag="wq", name="wqh0")
                            load_w_half(Wq_r, wq_h[0], 0)
                            wq_h[1] = p_wq.tile([P, n_dt, HW], BF16,
                                                tag="wq", name="wqh1")
                            load_w_half(Wq_r, wq_h[1], 1)
                    for qt in range(n_qt):
                        qs = slice(qt * P, (qt + 1) * P)
                        qt_g = c * n_qt + qt
                        pa = pp.tile([P, HW], F32, tag="ps", name="psA")
                        pb = pp.tile([P, HW], F32, tag="ps", name="psB")
                        pd = psd_p.tile([P, 8], F32, tag="psd", name="psd")
                        for kt in range(n_st):
                            st_ap = expT[:, kt, qs]
                            nc.tensor.matmul(pd[:], st_ap, ones_bf[:],
                                             start=(kt == 0), stop=(kt == n_st - 1))
                            nc.tensor.matmul(pa[:], st_ap, v2[:, kt, 0:HW],
                                             start=(kt == 0), stop=(kt == n_st - 1))
                            nc.tensor.matmul(pb[:], st_ap, v2[:, kt, HW:CW],
                                             start=(kt == 0), stop=(kt == n_st - 1))
                        rden = p_stat.tile([P, 1], F32, tag="rden", name="rden")
                        nc.vector.reciprocal(rden[:], pd[:, 0:1])
                        for half, ps in ((0, pa), (1, pb)):
                            osb = p_o.tile([P, HW], F32, tag="osb", name="osb")
                            nc.vector.tensor_scalar_mul(osb[:], ps[:],
                                                        rden[:, 0:1])
                            nc.gpsimd.dma_start(
                                out=out_r[:, qt_g, half * HW:(half + 1) * HW],
                                in_=osb[:])

    nc.compile()
    return nc


_NC_CACHE = {}


def _get_nc(S, D):
    if (S, D) not in _NC_CACHE:
        _NC_CACHE[(S, D)] = build(S, D)
    return _NC_CACHE[(S, D)]


def kernel(x1, x2, Wq, bq, Wk, bk, Wv, bv):
    B, S, D = x1.shape
    assert (B, S, D) == (8, 2048, 1024), (B, S, D)
    nc = _get_nc(S, D)
    f = np.float32
    shared = {
        "Wq": np.ascontiguousarray(Wq, f), "bq": np.ascontiguousarray(bq, f),
        "Wk": np.ascontiguousarray(Wk, f), "bk": np.ascontiguousarray(bk, f),
        "Wv": np.ascontiguousarray(Wv, f), "bv": np.ascontiguousarray(bv, f),
    }
    in_maps = [
        dict(x1=np.ascontiguousarray(x1[b], f),
             x2=np.ascontiguousarray(x2[b], f), **shared)
        for b in range(N_CORES)
    ]
    res = run_bass_kernel_spmd(nc, in_maps, list(range(N_CORES))).results
    return np.stack([res[b]["out"] for b in range(N_CORES)], axis=0).astype(f)
